# revision 23
# baseline (speedup 1.0000x reference)
"""ROI max-pooling (B=2, N=64, C=256, H=W=64, 7x7 out) on 8 TRN2 cores — v4.

Design (v4):
- Host converts conv_out to fp16 and pre-slices a per-core row band
  (uniform R_MAX rows across cores); core = b*4 + nh*2 + cg.
- All reduction on the DVE, with forms chosen for the DVE perf modes:
  * TensorTensor max gets 2x_1p (0.5 cyc/elem) when all operands are
    fp16 with a packed (stride-1, count>=2) innermost AP dim.
  * TensorCopy gets 2x_2p (SBUF) / 4x when also packed.
  * TensorReduce gets NO perf mode (1 cyc/elem) — avoid where a TT
    chain is cheaper.
  Forms:
  * kh=1,kw=1: single 4x copy slab->ostage.
  * kh=1,kw=2: single TT from slab.
  * kh=1,kw>=3: single reduce from slab.
  * kh>=2,kw=1 (sw==1 always in-dist): TT chain writing ostage
    directly (wext==7), no tmp, no horizontal stage.
  * kh>=2,kw>=2: vertical TT chain into a [7,wext] fp16 strip (always
    2x), then a horizontal stage from the strip. Horizontal stages of
    two ROIs with the same (sw,kw) are PAIRED: both strips go in one
    tmp and one instruction covers both output slots.
- Same-engine RAW chains (vertical steps, chained horizontals) are
  spaced by interleaving the two chains of a pair and by flushing the
  previous pair's horizontal into the gap; engine_nop as last resort.
- Output staged fp16, slot order = completion order, slice marks are
  uniform across bodies so SP/Act need no registers or branches.
"""

import os

os.environ.setdefault("MYCRO_LOCAL_CACHE", "1")

import numpy as np

B, N, C, H, W = 2, 64, 256, 64, 64
POOL_H = POOL_W = 7
ANCHOR_STRIDE = 16
N_CORES = 8
N_PER_CORE = N // 2  # 32
CELLS = POOL_H * POOL_W  # 49
N_CHUNKS = 4

# ---- DVE cost model (ns), calibrated from HW traces ------------------------
F_OP = 72.0  # per-instruction fixed busy (TT/copy; reduce ~62)
R1 = 1.042  # cyc/elem, no perf mode (reduce, strided TT)
R2 = 0.53  # 2x_1p TT / 2x_2p copy
R4 = 0.53  # copy (4x not observed on HW; treat as 2x)

# schedule model constants (ns), relative to NEFF t=0
_ROW_NS = 95.0  # fp16 row (128p x 64 x 2B = 16 KiB) per-queue transfer
_T_USER = 8300.0  # barrier release + first SP op
_T_ISSUE = 700.0  # DIRECT2D descriptor gen
_T_SEM = 950.0  # DGE start delay + completion sem propagation
_T_SETUP = 10250.0  # vector regs + branch resolved (abs)


def _wext(sw, kw):
    return 6 * sw + kw


def _roi_cost(kh, kw, sh, sw):
    """(cost_ns, n_ops) for one ROI, horizontal unpaired."""
    if kh == 1:
        if kw == 1:
            return (F_OP + R4 * 49, 1)
        if kw == 2:
            r = R2 if sw == 1 else R1
            return (F_OP + r * 49, 1)
        return (F_OP + R1 * 49 * kw, 1)
    if kw == 1:
        return ((kh - 1) * (F_OP + R2 * 49), kh - 1)
    we = _wext(sw, kw)
    v = (kh - 1) * (F_OP + R2 * 7 * we)
    h, nh = _h_cost(sw, kw, 1)
    return (v + h, kh - 1 + nh)


def _h_cost(sw, kw, npair):
    """(cost_ns, n_ops) of the horizontal stage covering npair strips."""
    e = 49 * npair
    if kw == 1:
        return (F_OP + R2 * e, 1)  # strided copy, 2x_2p
    if kw == 2:
        r = R2 if sw == 1 else R1
        return (F_OP + r * e, 1)
    if kw == 3:
        return (2 * (F_OP + R1 * e), 2)  # TT chain
    return (F_OP + R1 * e * kw, 1)  # kw=4: reduce


# ---- roi params ------------------------------------------------------------
def _expand(lo, hi, pool, limit):
    for _ in range(pool):
        need = (hi - lo + 1) < pool
        lo = np.where(need, np.maximum(0, lo - 1), lo)
        hi = np.where(need, np.minimum(limit - 1, hi + 1), hi)
    return lo, hi


def _roi_params(rois: np.ndarray):
    coords = (np.asarray(rois, np.float32) / ANCHOR_STRIDE).astype(np.int32)
    x1, y1, x2, y2 = (coords[..., i] for i in range(4))
    y1, y2 = _expand(y1, y2, POOL_H, H)
    x1, x2 = _expand(x1, x2, POOL_W, W)
    rh = y2 - y1 + 1
    rw = x2 - x1 + 1
    kh = -(-rh // POOL_H)
    sh = rh // POOL_H
    kw = -(-rw // POOL_W)
    sw = rw // POOL_W
    return y1, x1, sh, sw, kh, kw


# ---- planning --------------------------------------------------------------
def _row_extent(params, b, n):
    y1, x1, sh, sw, kh, kw = params
    lo = int(y1[b, n])
    hi = lo + 6 * int(sh[b, n]) + int(kh[b, n]) - 1
    return lo, hi


def _chunk_bounds(r_max, c0):
    rest = r_max - c0
    s1 = max(8, rest // 4)
    s2 = (rest - s1) // 2
    sizes = [c0, s1, s2, rest - s1 - s2]
    bounds = []
    acc = 0
    for s in sizes:
        acc += s
        bounds.append(acc)
    return bounds


def _c0_for(params, rs, metas, r_max):
    """Chunk-0 rows: enough that every body has >=1 ROI resident."""
    need = 0
    for rois, (bands, offs, rows) in zip(rs, metas):
        phis = sorted(
            offs[b] + _row_extent(params, b, n)[1] for b, n in rois
        )
        need = max(need, phis[0] + 1)
    return max(4, min(need, r_max // 2))


def _land_times(r_max, c0):
    """Modeled absolute landing time per chunk (2 queues, shared BW)."""
    bounds = _chunk_bounds(r_max, c0)
    sizes = [bounds[0]] + [bounds[i] - bounds[i - 1] for i in range(1, N_CHUNKS)]
    t = _T_USER + _T_ISSUE
    land = [0.0] * N_CHUNKS
    # q0: c0, c2; q1: c1, c3 — model aggregate 60% per queue when overlapped
    land[0] = t + sizes[0] * _ROW_NS + _T_SEM
    land[1] = t + (sizes[0] + sizes[1]) * 0.8 * _ROW_NS + _T_SEM
    land[2] = t + (sizes[0] + sizes[1] + sizes[2]) * 0.72 * _ROW_NS + _T_SEM
    land[3] = t + r_max * 0.7 * _ROW_NS + _T_SEM
    return land


def _chunk_of(row, r_max, c0):
    bounds = _chunk_bounds(r_max, c0)
    rel = min(max(row, 0), r_max - 1)
    for c, e in enumerate(bounds):
        if rel < e:
            return c
    return N_CHUNKS - 1


def _bands(params, rois):
    bands = {}
    for b, n in rois:
        lo, hi = _row_extent(params, b, n)
        if b in bands:
            bands[b] = (min(bands[b][0], lo), max(bands[b][1], hi))
        else:
            bands[b] = (lo, hi)
    rows = 0
    offs = {}
    for b in sorted(bands):
        offs[b] = rows - bands[b][0]
        rows += bands[b][1] - bands[b][0] + 1
    return bands, offs, rows


def _units_for(params, rois, offs, r_max, c0):
    """Build scheduling units: pairs of split rois (same sw,kw), plus
    solo splits, chains, ones. Returns list of dicts."""
    y1, x1, sh, sw, kh, kw = params

    def cneed(bn):
        b, n = bn
        lo, hi = _row_extent(params, b, n)
        return _chunk_of(offs[b] + hi, r_max, c0)

    splits, others = [], []
    for bn in rois:
        b, n = bn
        _kh, _kw = int(kh[b, n]), int(kw[b, n])
        if _kh >= 2 and _kw >= 2:
            splits.append(bn)
        else:
            others.append(bn)

    # pair splits by (sw,kw), preferring close chunk needs
    groups = {}
    for bn in splits:
        b, n = bn
        key = (int(sw[b, n]), int(kw[b, n]))
        groups.setdefault(key, []).append(bn)
    units = []
    for key, mem in groups.items():
        mem.sort(key=cneed)
        i = 0
        while i < len(mem):
            # group up to 4, but never span more than one chunk boundary
            jmax = i + 1
            while (
                jmax < len(mem)
                and jmax - i < 4
                and cneed(mem[jmax]) <= cneed(mem[i]) + (1 if jmax - i < 2 else 0)
            ):
                jmax += 1
            grp = mem[i:jmax]
            i = jmax
            if len(grp) == 1:
                units.append(dict(kind="solo", rois=grp, key=key))
            else:
                units.append(dict(kind="pair", rois=grp, key=key))
    for bn in others:
        b, n = bn
        _kh = int(kh[b, n])
        kind = "one" if _kh == 1 else "chain"
        units.append(dict(kind=kind, rois=[bn]))

    for u in units:
        u["cneed"] = max(cneed(bn) for bn in u["rois"])
        cost = 0.0
        for bn in u["rois"]:
            b, n = bn
            _kh, _kw, _sh, _sw = (
                int(kh[b, n]), int(kw[b, n]), int(sh[b, n]), int(sw[b, n]))
            if u["kind"] in ("pair", "solo"):
                we = _wext(_sw, _kw)
                cost += (_kh - 1) * (F_OP + R2 * 7 * we)
            else:
                cost += _roi_cost(_kh, _kw, _sh, _sw)[0]
        if u["kind"] in ("pair", "solo"):
            cost += _h_cost(*u["key"], len(u["rois"]))[0]
        u["cost"] = cost
    return units


def _sched_units(units, r_max, c0):
    """Order units by chunk readiness; return (makespan, ordered units)."""
    land = _land_times(r_max, c0)
    units = sorted(units, key=lambda u: (u["cneed"], -u["cost"]))
    clk = _T_SETUP
    for u in units:
        clk = max(clk, land[u["cneed"]]) + u["cost"]
    return clk, units


def _plan(params):
    bodies_rois = []
    for b in range(B):
        ext = [_row_extent(params, b, n) for n in range(N)]
        order = sorted(range(N), key=lambda n: ext[n][0] + ext[n][1])
        bodies_rois.append([(b, n) for n in order[:N_PER_CORE]])
        bodies_rois.append([(b, n) for n in order[N_PER_CORE:]])

    def score(rs):
        metas = [_bands(params, r) for r in rs]
        r_used = max(m[2] for m in metas)
        r_used = min(2 * H, -(-r_used // 4) * 4)
        c0 = _c0_for(params, rs, metas, r_used)
        mks = []
        for r, (bands, offs, rows) in zip(rs, metas):
            units = _units_for(params, r, offs, r_used, c0)
            mk, _ = _sched_units(units, r_used, c0)
            mks.append(mk)
        return max(mks) + 3.0 * r_used

    cur = score(bodies_rois)
    rng = np.random.default_rng(0)
    pairs = [(0, 1), (2, 3)] * 6 + [(0, 2), (1, 3), (0, 3), (1, 2)]
    for it in range(2000):
        if it % 3 < 2:
            j1, j2 = pairs[int(rng.integers(0, len(pairs)))]
        else:
            j1, j2 = int(rng.integers(0, 4)), int(rng.integers(0, 4))
            if j1 == j2:
                continue
        i1 = int(rng.integers(0, N_PER_CORE))
        i2 = int(rng.integers(0, N_PER_CORE))
        a, bq = bodies_rois[j1], bodies_rois[j2]
        a[i1], bq[i2] = bq[i2], a[i1]
        new = score(bodies_rois)
        if new <= cur:
            cur = new
        else:
            a[i1], bq[i2] = bq[i2], a[i1]

    metas = [_bands(params, r) for r in bodies_rois]
    r_max = max(m[2] for m in metas)
    r_max = min(2 * H, -(-r_max // 4) * 4)
    c0 = _c0_for(params, bodies_rois, metas, r_max)

    bodies = []
    for j in range(4):
        rois = bodies_rois[j]
        bands, offs, rows = metas[j]
        units = _units_for(params, rois, offs, r_max, c0)
        mk, order = _sched_units(units, r_max, c0)
        # slots in completion order
        slots = {}
        s = 0
        for u in order:
            for bn in u["rois"]:
                slots[bn] = s
                s += 1
        bodies.append(
            dict(rois=rois, bands=bands, offs=offs, units=order, slots=slots, mk=mk)
        )
    return bodies, r_max, c0


# ---- device program --------------------------------------------------------
MARKS = [12, 24, 30, 32]  # uniform completion-count slice marks


def _build_nc(params):
    import contextlib

    import concourse.bass as bass
    from concourse import mybir

    y1, x1, sh, sw, kh, kw = params
    f16 = mybir.dt.float16

    bodies, r_max, c0 = _plan(params)
    FS = r_max * W
    OS = N_PER_CORE * CELLS
    bounds = _chunk_bounds(r_max, c0)
    starts = [0] + bounds[:-1]

    branch_order = sorted(range(4), key=lambda j: -bodies[j]["mk"])

    nc = bass.Bass(monotonic_sem_count=0)
    conv = nc.declare_dram_parameter("conv", [128, FS], f16, isOutput=False)
    out = nc.declare_dram_parameter("out", [128, OS], f16, isOutput=True)

    with contextlib.ExitStack() as ctx:
        slab = ctx.enter_context(nc.sbuf_tensor("slab", [128, FS], f16))
        ostage = ctx.enter_context(nc.sbuf_tensor("ostage", [128, OS], f16))
        tmps = [
            ctx.enter_context(nc.sbuf_tensor(f"tmp{i}", [128, 4 * 7 * 22], f16))
            for i in range(4)
        ]
        chunk_sems = [
            ctx.enter_context(nc.semaphore(f"chunk{c}")) for c in range(N_CHUNKS)
        ]
        vsem = ctx.enter_context(nc.semaphore("vsem"))
        osem = ctx.enter_context(nc.semaphore("osem"))
        block = ctx.enter_context(nc.Block())

        sl = slab[:]
        slab_t = sl.tensor
        part_pair = list(sl.ap[0])

        def chunk_dma(eng, c):
            eng.dma_start(
                slab[:, starts[c] * W : bounds[c] * W],
                conv[:, starts[c] * W : bounds[c] * W],
            ).then_inc(chunk_sems[c], 16)

        def out_slice(eng, k):
            lo_s = 0 if k == 0 else MARKS[k - 1]
            hi_s = MARKS[k]
            eng.wait_ge(vsem, k + 1)
            eng.dma_start(
                out[:, lo_s * CELLS : hi_s * CELLS],
                ostage[:, lo_s * CELLS : hi_s * CELLS],
            ).then_inc(osem, 16)

        @block.sync
        def _(sync):
            chunk_dma(sync, 0)
            chunk_dma(sync, 2)
            out_slice(sync, 0)
            out_slice(sync, 2)

        @block.scalar
        def _(scalar):
            chunk_dma(scalar, 1)
            chunk_dma(scalar, 3)
            out_slice(scalar, 1)
            out_slice(scalar, 3)

        AluMax = mybir.AluOpType.max
        AxisX = mybir.AxisListType.X

        def emit_body(vector, j):
            # wrap compute methods to log (op, free_elems, packed) per emission
            dbg = DEBUG_OPS.setdefault(j, [])

            def _packed(ap):
                try:
                    last = ap.ap[-1]
                    return last[0] in (1, -1) and last[1] >= 2
                except Exception:
                    return False

            def _fs(ap):
                fs = 1
                for st, ct in list(ap.ap)[1:]:
                    fs *= ct
                return fs

            _tt, _rm, _tc, _nop = (vector.tensor_tensor, vector.reduce_max,
                                   vector.tensor_copy, vector.engine_nop)

            def tt(out, a, b_, **kw_):
                dbg.append(("tt", max(_fs(a), _fs(b_), _fs(out)),
                            _packed(a) and _packed(b_) and _packed(out)))
                return _tt(out, a, b_, **kw_)

            def rm(out, in_, **kw_):
                dbg.append(("red", max(_fs(in_), _fs(out)), False))
                return _rm(out, in_, **kw_)

            def tc(out, in_):
                dbg.append(("copy", max(_fs(in_), _fs(out)),
                            _packed(in_) and _packed(out)))
                return _tc(out, in_)

            def nop():
                dbg.append(("nop", 0, False))
                return _nop()

            vector = type("V", (), dict(
                tensor_tensor=staticmethod(tt), reduce_max=staticmethod(rm),
                tensor_copy=staticmethod(tc), engine_nop=staticmethod(nop),
                wait_ge=staticmethod(vector.wait_ge)))()

            bd = bodies[j]
            offs = bd["offs"]
            slots = bd["slots"]
            units = bd["units"]

            def slab_ap(b, n, dr, dc, inner):
                base = (
                    sl.offset
                    + (int(y1[b, n]) + offs[b] + dr) * W
                    + int(x1[b, n])
                    + dc
                )
                return bass.AP(slab_t, base, [part_pair] + inner)

            def slot_ap(s, count=1):
                return ostage[:, s * CELLS : (s + count) * CELLS]

            waited = set()
            done = 0
            mark_i = 0
            last_chain = [None]  # chain id of previously emitted op
            pending = []  # list of (chain_id, emit_fn, completes)

            def emit(chain_id, fn, completes=()):
                """Emit one op; flush pending H ops into gaps."""
                nonlocal done, mark_i
                inst = fn()
                last_chain[0] = chain_id
                _complete(inst, completes)
                return inst

            def _complete(inst, completes):
                nonlocal done, mark_i
                if not completes:
                    return
                done += len(completes)
                incs = 0
                while mark_i < len(MARKS) and done >= MARKS[mark_i]:
                    incs += 1
                    mark_i += 1
                if incs:
                    inst.then_inc(vsem, incs)

            def flush_pending(force=False):
                """Emit pending H ops whose dep chain differs from last op."""
                while pending:
                    cid, fn, comps = pending[0]
                    if cid == last_chain[0]:
                        if not force:
                            return
                        vector.engine_nop()
                        last_chain[0] = None
                    pending.pop(0)
                    inst = fn()
                    last_chain[0] = cid
                    _complete(inst, comps)

            tmp_i = 0
            for u in units:
                for cc in range(u["cneed"] + 1):
                    if cc not in waited:
                        vector.wait_ge(chunk_sems[cc], 16)
                        waited.add(cc)
                kind = u["kind"]
                if kind == "one":
                    b, n = u["rois"][0]
                    _kh, _kw, _sh, _sw = (
                        int(kh[b, n]), int(kw[b, n]),
                        int(sh[b, n]), int(sw[b, n]))
                    s = slots[(b, n)]
                    dims = [[_sh * W, 7], [_sw, 7]]
                    if _kw == 1:
                        emit((b, n), lambda b=b, n=n, s=s, dims=dims:
                             vector.tensor_copy(slot_ap(s), slab_ap(b, n, 0, 0, dims)),
                             [(b, n)])
                    elif _kw == 2:
                        emit((b, n), lambda b=b, n=n, s=s, dims=dims:
                             vector.tensor_tensor(
                                 slot_ap(s), slab_ap(b, n, 0, 0, dims),
                                 slab_ap(b, n, 0, 1, dims), op=AluMax),
                             [(b, n)])
                    else:
                        emit((b, n), lambda b=b, n=n, s=s, dims=dims, _kw=_kw:
                             vector.reduce_max(
                                 slot_ap(s),
                                 slab_ap(b, n, 0, 0, dims + [[1, _kw]]),
                                 axis=AxisX),
                             [(b, n)])
                    flush_pending()
                elif kind == "chain":
                    b, n = u["rois"][0]
                    _kh, _sh = int(kh[b, n]), int(sh[b, n])
                    s = slots[(b, n)]
                    dims = [[_sh * W, 7], [1, 7]]
                    emit((b, n), lambda b=b, n=n, s=s, dims=dims:
                         vector.tensor_tensor(
                             slot_ap(s), slab_ap(b, n, 0, 0, dims),
                             slab_ap(b, n, 1, 0, dims), op=AluMax),
                         [(b, n)] if _kh == 2 else ())
                    for d in range(2, _kh):
                        flush_pending(force=(last_chain[0] == (b, n)))
                        if last_chain[0] == (b, n):
                            vector.engine_nop()
                            last_chain[0] = None
                        emit((b, n), lambda b=b, n=n, s=s, dims=dims, d=d:
                             vector.tensor_tensor(
                                 slot_ap(s), slot_ap(s),
                                 slab_ap(b, n, d, 0, dims), op=AluMax),
                             [(b, n)] if d == _kh - 1 else ())
                    flush_pending()
                else:  # pair / solo split
                    _sw, _kw = u["key"]
                    we = _wext(_sw, _kw)
                    tmp = tmps[tmp_i % 4]
                    tmp_i += 1
                    mem = u["rois"]
                    khs = [int(kh[b, n]) for b, n in mem]
                    strip = []
                    for i2 in range(len(mem)):
                        tap = tmp[:]
                        strip.append(
                            bass.AP(tap.tensor, tap.offset + i2 * 7 * we,
                                    [list(tap.ap[0]), [we, 7], [1, we]]))
                    # verticals interleaved
                    for d in range(1, max(khs)):
                        for i2, (b, n) in enumerate(mem):
                            if d >= khs[i2]:
                                continue
                            _sh = int(sh[b, n])
                            dims = [[_sh * W, 7], [1, we]]
                            if d == 1:
                                emit((b, n), lambda b=b, n=n, i2=i2, dims=dims:
                                     vector.tensor_tensor(
                                         strip[i2], slab_ap(b, n, 0, 0, dims),
                                         slab_ap(b, n, 1, 0, dims), op=AluMax))
                            else:
                                if last_chain[0] == (b, n):
                                    flush_pending(force=False)
                                if last_chain[0] == (b, n):
                                    vector.engine_nop()
                                    last_chain[0] = None
                                emit((b, n), lambda b=b, n=n, i2=i2, dims=dims, d=d:
                                     vector.tensor_tensor(
                                         strip[i2], strip[i2],
                                         slab_ap(b, n, d, 0, dims), op=AluMax))
                        flush_pending()
                    # horizontal: pending, flushed >=1 op later.
                    # hcid == last member (b,n): the spacing check then
                    # guards the RAW on that member's freshly written strip.
                    npair = len(mem)
                    tap = tmp[:]
                    s0 = min(slots[bn] for bn in mem)
                    assert [slots[bn] for bn in mem] == list(
                        range(s0, s0 + npair))
                    hcid = mem[-1]

                    def mk_hsrc(tap_t=tap.tensor, tap_off=tap.offset,
                                tpart=tuple(tap.ap[0]), we=we, sw_=_sw,
                                npair=npair):
                        def hsrc(dc, extra=None):
                            dims = ([[7 * we, npair]] if npair >= 2 else []) \
                                + [[we, 7], [sw_, 7]]
                            if extra:
                                dims = dims + [extra]
                            return bass.AP(tap_t, tap_off + dc,
                                           [list(tpart)] + dims)
                        return hsrc

                    hsrc = mk_hsrc()
                    out2 = slot_ap(s0, npair)
                    if _kw == 1:
                        pending.append((hcid, lambda out2=out2, hsrc=hsrc:
                                        vector.tensor_copy(out2, hsrc(0)),
                                        list(mem)))
                    elif _kw == 2:
                        pending.append((hcid, lambda out2=out2, hsrc=hsrc:
                                        vector.tensor_tensor(
                                            out2, hsrc(0), hsrc(1), op=AluMax),
                                        list(mem)))
                    elif _kw == 3:
                        pending.append((hcid, lambda out2=out2, hsrc=hsrc:
                                        vector.tensor_tensor(
                                            out2, hsrc(0), hsrc(1), op=AluMax),
                                        ()))
                        pending.append((hcid, lambda out2=out2, hsrc=hsrc:
                                        vector.tensor_tensor(
                                            out2, out2, hsrc(2), op=AluMax),
                                        list(mem)))
                    else:
                        pending.append((hcid, lambda out2=out2, hsrc=hsrc,
                                        _kw=_kw:
                                        vector.reduce_max(
                                            out2, hsrc(0, [1, _kw]), axis=AxisX),
                                        list(mem)))
            flush_pending(force=True)
            if mark_i < len(MARKS):
                raise RuntimeError("marks not all reached")

        @block.vector
        def _(vector):
            pid = vector.alloc_register("pid")
            vector.reg_load(pid, nc.partition_id_tensor[0:1, 0:1])

            # balanced dispatch: 2 branches deep for every core
            with vector.If_lt(pid, 4):
                with vector.If_lt(pid, 2):
                    emit_body(vector, 0)
                with vector.Else():
                    emit_body(vector, 1)
            with vector.Else():
                with vector.If_lt(pid, 6):
                    emit_body(vector, 2)
                with vector.Else():
                    emit_body(vector, 3)

    return nc, bodies, r_max


_CACHE: dict[bytes, object] = {}
LAST_RESULT = None
LAST_PLAN = None
DEBUG_OPS: dict[int, list] = {}


def _get_built(params_key: bytes, params):
    built = _CACHE.get(params_key)
    if built is None:
        built = _build_nc(params)
        _CACHE[params_key] = built
    return built


def kernel(rois: np.ndarray, conv_out: np.ndarray) -> np.ndarray:
    from concourse.bass_utils import run_bass_kernel_spmd

    rois = np.asarray(rois)
    conv_out = np.asarray(conv_out, np.float32)
    params = _roi_params(rois)
    params_key = b"".join(np.ascontiguousarray(p).tobytes() for p in params)
    nc, bodies, r_max = _get_built(params_key, params)
    global LAST_PLAN
    LAST_PLAN = (bodies, r_max)

    in_maps = []
    for core in range(N_CORES):
        j, cg = core >> 1, core & 1
        bd = bodies[j]
        slab = np.zeros((128, r_max, W), np.float16)
        for b, (lo, hi) in bd["bands"].items():
            off = bd["offs"][b] + lo
            slab[:, off : off + hi - lo + 1] = conv_out[
                b, cg * 128 : (cg + 1) * 128, lo : hi + 1, :
            ]
        in_maps.append({"conv": slab.reshape(128, -1)})

    res = run_bass_kernel_spmd(nc, in_maps, list(range(N_CORES)))
    global LAST_RESULT
    LAST_RESULT = res

    out = np.empty((B, N, C, POOL_H, POOL_W), np.float32)
    for core in range(N_CORES):
        j, cg = core >> 1, core & 1
        bd = bodies[j]
        r = (
            res.results[core]["out"]
            .reshape(128, N_PER_CORE, CELLS)
            .astype(np.float32)
        )
        for b, n in bd["rois"]:
            s = bd["slots"][(b, n)]
            out[b, n, cg * 128 : (cg + 1) * 128] = r[:, s].reshape(
                128, POOL_H, POOL_W
            )
    return out
